# revision 10
# baseline (speedup 1.0000x reference)
"""Trainium2 Bass kernel for DiscriminatorAugment (translation + color jitter +
cutout), data-parallel over 8 NeuronCores (8 samples each).

Math: with x0 = translated image, the reference's color jitter chain
    x1 = x0 + badd;  x2 = (x1 - mean_c x1)*s + mean_c x1;
    x3 = (x2 - mean_chw x2)*t + mean_chw x2
collapses to the per-pixel affine
    x3 = A*x0 + BC*m3 + C,   A = t*s, BC = t*(1-s)/3, m3 = sum_c x0,
    C = (1-t)*g0 + badd,     g0 = (sum_chw x0)/(3*H*W)
and cutout multiplies by (1 - rowmask*colmask).

Device work per sample (software-pipelined load(b) | m3(b-1) | out(b-2)):
dynamic-offset DMA load of the shifted window from a zero-padded copy of the
input (= translation, channels split across both HWDGE queues), DVE adds +
fused row-sum for m3, PE matmul with ones for the cross-partition sum
broadcast, ACT for C and the D = BC*m3 + C tile, DVE scalar_tensor_tensor
for A*x + D, a CW-wide dynamically-positioned window multiply for cutout
(alternating DVE/GpSimd), stores mostly via GpSimd/SWDGE so the load queues
never stall behind compute waits. HW exec ~157-165us/core vs a ~134us
DMA floor (52MB at the ~390GB/s per-core HBM ceiling).
"""
import threading

import numpy as np

import concourse.bass as bass
import concourse.mybir as mybir
import concourse.tile as tile
from concourse.bass_utils import run_bass_kernel_spmd

M = 8          # cores
B = 64         # full batch
BS = B // M    # samples per core
C, H, W = 3, 512, 512
PAD = 64       # translation margin (delta_h = delta_w = 64)
HP, WP = H + 2 * PAD, W + 2 * PAD
P = 128
NJ = H // P    # 4 row-chunks of 128
CH = round(H * 0.2)   # 102 cutout rows
CW = 106              # static cutout column window, even start (covers any
                      # clipped range even after rounding the start down)
F32 = mybir.dt.float32
F16 = mybir.dt.float16
I32 = mybir.dt.int32

# pf columns
I_A, I_BC, I_GS, I_BADD = 0, 1, 2, 3


def _split_waits(nc, max_waits=1):
    """Walrus in this container rejects >2 sem waits on one instruction
    ("Too many sync wait commands"). Hoist excess waits onto standalone
    single-wait event-semaphore instructions immediately before, same
    engine — semantics identical (waits execute before the instruction
    in program order either way)."""
    uid = 0
    for f in nc.m.functions:
        for bb in f.blocks:
            new_list, changed = [], False
            for inst in bb.instructions:
                si = inst.sync_info
                waits = list(si.on_wait) if si and si.on_wait else []
                if len(waits) > max_waits:
                    changed = True
                    for w in waits[:-max_waits]:
                        uid += 1
                        ev = mybir.InstEventSemaphore(name=f"splitwait_{uid}")
                        ev.engine = inst.engine
                        ev.sync_info = mybir.SyncInfo(on_wait=[w], on_update=[])
                        new_list.append(ev)
                    inst.sync_info = mybir.SyncInfo(
                        on_wait=waits[-max_waits:],
                        on_update=list(si.on_update) if si.on_update else [],
                    )
                new_list.append(inst)
            if changed:
                bb.instructions = new_list


def _bcast_part(ap, p=P):
    """Replicate a DRAM AP across p partitions (0-stride partition dim)."""
    return bass.AP(tensor=ap.tensor, offset=ap.offset, ap=[[0, p]] + list(ap.ap))


def _build_program():
    nc = bass.Bass(num_swdge_queues=4)
    img = nc.declare_dram_parameter("img", [BS, C, HP, WP], F16, isOutput=False)
    pf = nc.declare_dram_parameter("pf", [BS, 4], F32, isOutput=False)
    pi = nc.declare_dram_parameter("pi", [BS, 1], I32, isOutput=False)
    pcs = nc.declare_dram_parameter("pcs", [BS, 1], I32, isOutput=False)
    invw = nc.declare_dram_parameter("invw", [BS, H, CW], F16, isOutput=False)
    out = nc.declare_dram_parameter("out", [BS, C, H, W], F16, isOutput=True)

    Alu = mybir.AluOpType
    Act = mybir.ActivationFunctionType
    SP = mybir.EngineType.SP

    with tile.TileContext(nc) as tc:
        with (
            tc.tile_pool(name="work", bufs=5) as work,
            tc.tile_pool(name="singles", bufs=1) as singles,
            tc.tile_pool(name="psum", bufs=4, space="PSUM") as psum,
        ):
            ones_t = singles.tile([P, P], F32)
            nc.vector.memset(ones_t[:], 1.0)
            pf_sb = singles.tile([P, BS, 4], F32)
            nc.scalar.dma_start(out=pf_sb[:], in_=_bcast_part(pf[:]))
            # stage the dynamic offsets in SBUF: register loads from DRAM
            # take ~2-3us on the issuing engine, from SBUF they are cheap
            pi_sb = singles.tile([1, BS], I32)
            nc.sync.dma_start(out=pi_sb[:], in_=pi[:].rearrange("b one -> one b"))
            pcs_sb = singles.tile([1, BS], I32)
            nc.scalar.dma_start(out=pcs_sb[:], in_=pcs[:].rearrange("b one -> one b"))

            state = {}

            def stage_load(b):
                # alternate the HWDGE issuing engine for the dynamic loads:
                # each engine's register file only fits ~half the samples'
                # dynamic-offset expressions
                ld_eng_t = SP if b % 2 == 0 else mybir.EngineType.Activation
                ld_eng = nc.sync if b % 2 == 0 else nc.scalar
                x_t = work.tile([P, C, NJ, W], F16, tag="x")
                invw_t = work.tile([P, 1, NJ, CW], F16, tag="invw")
                # translated window load (dynamic element offset from pi:
                # pi[b] = r0*WP + c0), channels split across BOTH HWDGE
                # engines/queues so each sample's load latency halves.
                # During the fill the SWDGE store queue is idle, so the
                # first two samples also use it for the middle channel.
                engines = [SP, mybir.EngineType.Activation]
                if b < 2:
                    engines.append(mybir.EngineType.Pool)
                off = nc.values_load(
                    pi_sb[0:1, b : b + 1],
                    engines=engines,
                    min_val=0,
                    max_val=(HP - H) * WP + (WP - W),
                    skip_runtime_bounds_check=True,
                )
                for c in range(C):
                    base = img[b, c]
                    src = bass.AP(
                        tensor=base.tensor,
                        offset=base.offset + off,
                        ap=[[WP, P], [P * WP, NJ], [1, W]],
                    )
                    if c == 1:
                        eng = nc.gpsimd if b < 2 else ld_eng
                    else:
                        eng = nc.scalar if b % 2 == 0 else nc.sync
                    eng.dma_start(out=x_t[:, c], in_=src)
                (nc.gpsimd if b < 2 else ld_eng).dma_start(
                    out=invw_t[:, 0],
                    in_=invw[b].rearrange("(j p) w -> p j w", p=P),
                )
                state[b] = dict(x_t=x_t, invw_t=invw_t)

            def stage_m3(b):
                st = state[b]
                x_t, invw_t = st["x_t"], st["invw_t"]
                m3_t = work.tile([P, 1, NJ, W], F16, tag="m3")
                s_t = work.tile([P, 1], F32, tag="s")
                c_t = work.tile([P, 1], F32, tag="c")
                g_t = psum.tile([P, 1], F32, tag="g")
                # x' = A*x in place over all channels (tensor_scalar runs in
                # 4x DVE perf mode, unlike scalar_tensor_tensor which has no
                # accelerated uops), with the fused row-sum giving
                # s = rowsum(A*(x0+x1+x2)) = rowsum of m3' for free
                nc.vector.tensor_scalar(
                    out=x_t[:],
                    in0=x_t[:],
                    scalar1=pf_sb[:, b, I_A : I_A + 1],
                    scalar2=0.0,
                    op0=Alu.mult,
                    op1=Alu.add,
                    accum_out=s_t[:],
                )
                # m3' = x'0+x'1+x'2 (2x mode tensor_tensor adds)
                nc.vector.tensor_tensor(
                    m3_t[:, 0], x_t[:, 0], x_t[:, 1], Alu.add
                )
                nc.vector.tensor_tensor(
                    m3_t[:, 0], m3_t[:, 0], x_t[:, 2], Alu.add
                )
                # cross-partition sum of s, broadcast to all partitions
                nc.tensor.matmul(g_t[:], ones_t[:], s_t[:], start=True, stop=True)
                # C = (GS/A) * total' + badd   (per-partition [P,1]);
                # total' = A*total so this equals GS*total + badd
                nc.scalar.activation(
                    c_t[:],
                    g_t[:],
                    Act.Identity,
                    bias=pf_sb[:, b, I_BADD : I_BADD + 1],
                    scale=pf_sb[:, b, I_GS : I_GS + 1],
                )
                # u = (BC/A)*m3' + C  (in place over m3; equals BC*m3 + C)
                nc.scalar.activation(
                    m3_t[:, 0],
                    m3_t[:, 0],
                    Act.Identity,
                    bias=c_t[:],
                    scale=pf_sb[:, b, I_BC : I_BC + 1],
                )
                st["m3_t"] = m3_t
            def stage_out(b):
                st = state.pop(b)
                x_t, m3_t, invw_t = st["x_t"], st["m3_t"], st["invw_t"]
                # cutout mask multiply covers a CW-wide window at dynamic
                # start cs (host: min(b0, W-CW), always covers the
                # rectangle). Alternate DVE/gpsimd for register pressure.
                ap_eng_t = (
                    mybir.EngineType.DVE if b % 2 == 0 else mybir.EngineType.Pool
                )
                ap_eng = nc.vector if b % 2 == 0 else nc.gpsimd
                cs = nc.values_load(
                    pcs_sb[0:1, b : b + 1],
                    engines=[ap_eng_t],
                    min_val=0,
                    max_val=W - CW,
                    skip_runtime_bounds_check=True,
                )
                # out = x' + u, one 2x-mode tensor_tensor over all three
                # channels with u broadcast across the channel dim
                nc.vector.tensor_tensor(
                    x_t[:],
                    x_t[:],
                    m3_t[:].broadcast_to([P, C, NJ, W]),
                    Alu.add,
                )
                xwin = x_t[:, :, :, bass.ds(cs, CW)]
                ap_eng.tensor_tensor(
                    xwin, xwin, invw_t[:].broadcast_to([P, C, NJ, CW]), Alu.mult
                )
                # stores: two channels on gpsimd (SWDGE q0, never blocks the
                # load engines), one on an alternating HWDGE queue for
                # byte balance. Last two samples spread across all three.
                for c in range(C):
                    if b >= BS - 2:
                        st_eng = (nc.gpsimd, nc.sync, nc.scalar)[(b + c) % 3]
                    elif c == 2:
                        st_eng = nc.scalar if b % 2 == 0 else nc.sync
                    else:
                        st_eng = nc.gpsimd
                    st_eng.dma_start(
                        out=out[b, c].rearrange("(j p) w -> p j w", p=P),
                        in_=x_t[:, c],
                    )

            # software-pipelined emission: load(b) | m3(b-1) | out(b-2) so
            # the scheduler interleaves sample b+1's DVE work into sample
            # b's PE/ACT latency chain
            for i in range(BS + 2):
                if i < BS:
                    stage_load(i)
                if 0 <= i - 1 < BS:
                    stage_m3(i - 1)
                if 0 <= i - 2 < BS:
                    stage_out(i - 2)

    _split_waits(nc)
    return nc


_cache = threading.local()


def _get_program():
    nc = getattr(_cache, "nc", None)
    if nc is None:
        nc = _build_program()
        _cache.nc = nc
    return nc


def _host_params(images, rand01):
    """Per-sample parameters, computed with float32 semantics matching the
    jax reference."""
    r = np.asarray(rand01, dtype=np.float32).reshape(7, B)
    th = np.floor(r[0] * np.float32(2 * PAD + 1)).astype(np.int32) - PAD
    tw = np.floor(r[1] * np.float32(2 * PAD + 1)).astype(np.int32) - PAD
    badd = r[2] - np.float32(0.5)
    s = r[3] * np.float32(2.0)
    t = r[4] + np.float32(0.5)
    ch = round(H * 0.2)  # 102
    cw = round(W * 0.2)
    oh = np.floor(r[5] * np.float32(H + (1 - ch % 2))).astype(np.int32)
    ow = np.floor(r[6] * np.float32(W + (1 - cw % 2))).astype(np.int32)

    A = t * s
    BC = t * (np.float32(1.0) - s) / np.float32(3.0)
    GS = (np.float32(1.0) - t) / np.float32(3 * H * W)
    # the device scales x by A first (x' = A*x), so the m3'/total'-based
    # constants are pre-divided by A on the host
    pf = np.stack([A, BC / A, GS / A, badd], axis=1).astype(np.float32)  # [B,4]
    # fused element offset of the translated window within img[b, c]
    pi = ((th + PAD).astype(np.int64) * WP + (tw + PAD)).astype(np.int32)[
        :, None
    ]  # [B,1]

    idx = np.arange(H)
    a0 = np.maximum(0, oh - ch // 2)[:, None]
    a1 = np.minimum(H - 1, oh + (ch - ch // 2) - 1)[:, None]
    b0 = np.maximum(0, ow - cw // 2)[:, None]
    b1 = np.minimum(W - 1, ow + (cw - cw // 2) - 1)[:, None]
    rowz = (idx[None, :] >= a0) & (idx[None, :] <= a1)  # [B,H]
    colz = (idx[None, :] >= b0) & (idx[None, :] <= b1)  # [B,W]
    # even window start so the dynamic fp16 column slice stays 4B-aligned
    # (keeps the DVE cutout multiply in 2x perf mode)
    pcs0 = np.minimum(b0[:, 0], W - CW)
    pcs = (pcs0 - (pcs0 % 2)).astype(np.int32)[:, None]  # [B,1]
    # inverse cutout mask on the CW-wide window starting at pcs
    wi = pcs + np.arange(CW)[None, :]  # [B,CW]
    colz_win = np.take_along_axis(colz, wi, axis=1)  # [B,CW]
    invw = (
        1.0 - rowz[:, :, None] * colz_win[:, None, :]
    ).astype(np.float16)  # [B,H,CW]

    imp = np.zeros((B, C, HP, WP), dtype=np.float16)
    imp[:, :, PAD : PAD + H, PAD : PAD + W] = images
    return imp, pf, pi, pcs, invw


def _run(images, rand01, trace=False):
    images = np.ascontiguousarray(np.asarray(images, dtype=np.float32))
    imp, pf, pi, pcs, invw = _host_params(images, rand01)
    nc = _get_program()
    in_maps = [
        {
            "img": np.ascontiguousarray(imp[k * BS : (k + 1) * BS]),
            "pf": np.ascontiguousarray(pf[k * BS : (k + 1) * BS]),
            "pi": np.ascontiguousarray(pi[k * BS : (k + 1) * BS]),
            "pcs": np.ascontiguousarray(pcs[k * BS : (k + 1) * BS]),
            "invw": np.ascontiguousarray(invw[k * BS : (k + 1) * BS]),
        }
        for k in range(M)
    ]
    res = run_bass_kernel_spmd(nc, in_maps, list(range(M)), trace=trace)
    full = np.concatenate(
        [np.asarray(res.results[k]["out"], dtype=np.float32) for k in range(M)],
        axis=0,
    )
    return full, res


def kernel(images, rand01):
    full, _ = _run(images, rand01, trace=False)
    return full



# revision 13
# speedup vs baseline: 1.0978x; 1.0978x over previous
"""Trainium2 Bass kernel for DiscriminatorAugment (translation + color jitter +
cutout), data-parallel over 8 NeuronCores (8 samples each).

Math: with x0 = translated image, the reference's color jitter chain
    x1 = x0 + badd;  x2 = (x1 - mean_c x1)*s + mean_c x1;
    x3 = (x2 - mean_chw x2)*t + mean_chw x2
collapses to the per-pixel affine
    x3 = A*x0 + BC*m3 + C,   A = t*s, BC = t*(1-s)/3, m3 = sum_c x0,
    C = (1-t)*g0 + badd,     g0 = (sum_chw x0)/(3*H*W)
and cutout multiplies by (1 - rowmask*colmask).

Device work per sample (software-pipelined load(b) | m3(b-1) | out(b-2)):
dynamic-offset DMA load of the shifted window from a zero-padded copy of the
input (= translation, channels split across both HWDGE queues), DVE adds +
fused row-sum for m3, PE matmul with ones for the cross-partition sum
broadcast, ACT for C and the D = BC*m3 + C tile, DVE scalar_tensor_tensor
for A*x + D, a CW-wide dynamically-positioned window multiply for cutout
(alternating DVE/GpSimd), stores mostly via GpSimd/SWDGE so the load queues
never stall behind compute waits. HW exec ~157-165us/core vs a ~134us
DMA floor (52MB at the ~390GB/s per-core HBM ceiling).
"""
import threading

import numpy as np

import concourse.bass as bass
import concourse.mybir as mybir
import concourse.tile as tile
from concourse.bass_utils import run_bass_kernel_spmd

M = 8          # cores
B = 64         # full batch
BS = B // M    # samples per core
C, H, W = 3, 512, 512
PAD = 64       # translation margin (delta_h = delta_w = 64)
HP, WP = H + 2 * PAD, W + 2 * PAD
P = 128
NJ = H // P    # 4 row-chunks of 128
CH = round(H * 0.2)   # 102 cutout rows
CW = 106              # static cutout column window, even start (covers any
                      # clipped range even after rounding the start down)
F32 = mybir.dt.float32
F16 = mybir.dt.float16
I32 = mybir.dt.int32

# pf columns
I_A, I_BC, I_GS, I_BADD = 0, 1, 2, 3


def _split_waits(nc, max_waits=1):
    """Walrus in this container rejects >2 sem waits on one instruction
    ("Too many sync wait commands"). Hoist excess waits onto standalone
    single-wait event-semaphore instructions immediately before, same
    engine — semantics identical (waits execute before the instruction
    in program order either way)."""
    uid = 0
    for f in nc.m.functions:
        for bb in f.blocks:
            new_list, changed = [], False
            for inst in bb.instructions:
                si = inst.sync_info
                waits = list(si.on_wait) if si and si.on_wait else []
                if len(waits) > max_waits:
                    changed = True
                    for w in waits[:-max_waits]:
                        uid += 1
                        ev = mybir.InstEventSemaphore(name=f"splitwait_{uid}")
                        ev.engine = inst.engine
                        ev.sync_info = mybir.SyncInfo(on_wait=[w], on_update=[])
                        new_list.append(ev)
                    inst.sync_info = mybir.SyncInfo(
                        on_wait=waits[-max_waits:],
                        on_update=list(si.on_update) if si.on_update else [],
                    )
                new_list.append(inst)
            if changed:
                bb.instructions = new_list


def _bcast_part(ap, p=P):
    """Replicate a DRAM AP across p partitions (0-stride partition dim)."""
    return bass.AP(tensor=ap.tensor, offset=ap.offset, ap=[[0, p]] + list(ap.ap))


def _build_program():
    nc = bass.Bass(num_swdge_queues=4)
    img = nc.declare_dram_parameter("img", [BS, C, HP, WP], F16, isOutput=False)
    pf = nc.declare_dram_parameter("pf", [BS, 4], F32, isOutput=False)
    pi = nc.declare_dram_parameter("pi", [BS, 1], I32, isOutput=False)
    pcs = nc.declare_dram_parameter("pcs", [BS, 1], I32, isOutput=False)
    invw = nc.declare_dram_parameter("invw", [BS, H, CW], F16, isOutput=False)
    out = nc.declare_dram_parameter("out", [BS, C, H, W], F16, isOutput=True)

    Alu = mybir.AluOpType
    Act = mybir.ActivationFunctionType
    SP = mybir.EngineType.SP

    with tile.TileContext(nc) as tc:
        with (
            tc.tile_pool(name="work", bufs=5) as work,
            tc.tile_pool(name="singles", bufs=1) as singles,
            tc.tile_pool(name="psum", bufs=4, space="PSUM") as psum,
        ):
            ones_t = singles.tile([P, P], F32)
            nc.vector.memset(ones_t[:], 1.0)
            ones16_t = singles.tile([P, 1], F16)
            nc.vector.memset(ones16_t[:], 1.0)
            pf_sb = singles.tile([P, BS, 4], F32)
            nc.scalar.dma_start(out=pf_sb[:], in_=_bcast_part(pf[:]))
            # stage the dynamic offsets in SBUF: register loads from DRAM
            # take ~2-3us on the issuing engine, from SBUF they are cheap
            pi_sb = singles.tile([1, BS], I32)
            nc.sync.dma_start(out=pi_sb[:], in_=pi[:].rearrange("b one -> one b"))
            pcs_sb = singles.tile([1, BS], I32)
            nc.scalar.dma_start(out=pcs_sb[:], in_=pcs[:].rearrange("b one -> one b"))

            state = {}

            def stage_load(b):
                # alternate the HWDGE issuing engine for the dynamic loads:
                # each engine's register file only fits ~half the samples'
                # dynamic-offset expressions
                ld_eng_t = SP if b % 2 == 0 else mybir.EngineType.Activation
                ld_eng = nc.sync if b % 2 == 0 else nc.scalar
                x_t = work.tile([P, C, NJ, W], F16, tag="x")
                invw_t = work.tile([P, 1, NJ, CW], F16, tag="invw")
                # translated window load (dynamic element offset from pi:
                # pi[b] = r0*WP + c0), channels split across BOTH HWDGE
                # engines/queues so each sample's load latency halves.
                # During the fill the SWDGE store queue is idle, so the
                # first two samples also use it for the middle channel.
                engines = [SP, mybir.EngineType.Activation]
                if b < 2:
                    engines.append(mybir.EngineType.Pool)
                off = nc.values_load(
                    pi_sb[0:1, b : b + 1],
                    engines=engines,
                    min_val=0,
                    max_val=(HP - H) * WP + (WP - W),
                    skip_runtime_bounds_check=True,
                )
                for c in range(C):
                    base = img[b, c]
                    src = bass.AP(
                        tensor=base.tensor,
                        offset=base.offset + off,
                        ap=[[WP, P], [P * WP, NJ], [1, W]],
                    )
                    if c == 1:
                        eng = nc.gpsimd if b < 2 else ld_eng
                    else:
                        eng = nc.scalar if b % 2 == 0 else nc.sync
                    eng.dma_start(out=x_t[:, c], in_=src)
                (nc.gpsimd if b < 2 else ld_eng).dma_start(
                    out=invw_t[:, 0],
                    in_=invw[b].rearrange("(j p) w -> p j w", p=P),
                )
                state[b] = dict(x_t=x_t, invw_t=invw_t)

            def stage_m3(b):
                st = state[b]
                x_t = st["x_t"]
                m3_t = work.tile([P, 1, NJ, W], F16, tag="m3")
                c_t = work.tile([P, 1], F32, tag="c")
                tot_t = work.tile([1, 1], F32, tag="tot")
                cs_t = psum.tile([1, W], F32, tag="cs")
                g_t = psum.tile([P, 1], F32, tag="g")
                # m3 = x0+x1+x2 (2x mode tensor_tensor adds on raw x)
                nc.vector.tensor_tensor(
                    m3_t[:, 0], x_t[:, 0], x_t[:, 1], Alu.add
                )
                nc.vector.tensor_tensor(
                    m3_t[:, 0], m3_t[:, 0], x_t[:, 2], Alu.add
                )
                # global sum of m3 without touching DVE: PE column-sums the
                # four 512-wide blocks into one accumulating PSUM row, ACT
                # mini-reduces that row to a scalar, PE broadcasts it back
                # across all 128 partitions
                for k in range(NJ):
                    nc.tensor.matmul(
                        cs_t[:],
                        ones16_t[:],
                        m3_t[:, 0, k],
                        start=(k == 0),
                        stop=(k == NJ - 1),
                    )
                nc.scalar.activation(
                    cs_t[:], cs_t[:], Act.Identity, accum_out=tot_t[:]
                )
                nc.tensor.matmul(
                    g_t[:], ones_t[0:1, :], tot_t[:], start=True, stop=True
                )
                # C' = (GS/A) * total + badd/A   (per-partition [P,1])
                nc.scalar.activation(
                    c_t[:],
                    g_t[:],
                    Act.Identity,
                    bias=pf_sb[:, b, I_BADD : I_BADD + 1],
                    scale=pf_sb[:, b, I_GS : I_GS + 1],
                )
                # u' = (BC/A)*m3 + C'  (in place over m3), so that
                # A*(x + u') = A*x + BC*m3 + C exactly as the reference
                nc.scalar.activation(
                    m3_t[:, 0],
                    m3_t[:, 0],
                    Act.Identity,
                    bias=c_t[:],
                    scale=pf_sb[:, b, I_BC : I_BC + 1],
                )
                st["m3_t"] = m3_t
            def stage_out(b):
                st = state.pop(b)
                x_t, m3_t, invw_t = st["x_t"], st["m3_t"], st["invw_t"]
                # cutout mask multiply covers a CW-wide window at dynamic
                # start cs (host: min(b0, W-CW), always covers the
                # rectangle). Alternate DVE/gpsimd for register pressure.
                ap_eng_t = (
                    mybir.EngineType.DVE if b % 2 == 0 else mybir.EngineType.Pool
                )
                ap_eng = nc.vector if b % 2 == 0 else nc.gpsimd
                cs = nc.values_load(
                    pcs_sb[0:1, b : b + 1],
                    engines=[ap_eng_t],
                    min_val=0,
                    max_val=W - CW,
                    skip_runtime_bounds_check=True,
                )
                # y = x + u', one 2x-mode tensor_tensor over all three
                # channels with u' broadcast across the channel dim
                nc.vector.tensor_tensor(
                    x_t[:],
                    x_t[:],
                    m3_t[:].broadcast_to([P, C, NJ, W]),
                    Alu.add,
                )
                # out = A*y, plain tensor_scalar (4x perf mode)
                nc.vector.tensor_scalar(
                    out=x_t[:],
                    in0=x_t[:],
                    scalar1=pf_sb[:, b, I_A : I_A + 1],
                    scalar2=None,
                    op0=Alu.mult,
                )
                xwin = x_t[:, :, :, bass.ds(cs, CW)]
                ap_eng.tensor_tensor(
                    xwin, xwin, invw_t[:].broadcast_to([P, C, NJ, CW]), Alu.mult
                )
                # stores: two channels on gpsimd (SWDGE q0, never blocks the
                # load engines), one on an alternating HWDGE queue for
                # byte balance. Last two samples spread across all three.
                for c in range(C):
                    if b >= BS - 2:
                        st_eng = (nc.gpsimd, nc.sync, nc.scalar)[(b + c) % 3]
                    elif c == 2:
                        st_eng = nc.scalar if b % 2 == 0 else nc.sync
                    else:
                        st_eng = nc.gpsimd
                    st_eng.dma_start(
                        out=out[b, c].rearrange("(j p) w -> p j w", p=P),
                        in_=x_t[:, c],
                    )

            # software-pipelined emission: load(b) | m3(b-1) | out(b-2) so
            # the scheduler interleaves sample b+1's DVE work into sample
            # b's PE/ACT latency chain
            for i in range(BS + 2):
                if i < BS:
                    stage_load(i)
                if 0 <= i - 1 < BS:
                    stage_m3(i - 1)
                if 0 <= i - 2 < BS:
                    stage_out(i - 2)

    _split_waits(nc)
    return nc


_cache = threading.local()


def _get_program():
    nc = getattr(_cache, "nc", None)
    if nc is None:
        nc = _build_program()
        _cache.nc = nc
    return nc


def _host_params(images, rand01):
    """Per-sample parameters, computed with float32 semantics matching the
    jax reference."""
    r = np.asarray(rand01, dtype=np.float32).reshape(7, B)
    th = np.floor(r[0] * np.float32(2 * PAD + 1)).astype(np.int32) - PAD
    tw = np.floor(r[1] * np.float32(2 * PAD + 1)).astype(np.int32) - PAD
    badd = r[2] - np.float32(0.5)
    s = r[3] * np.float32(2.0)
    t = r[4] + np.float32(0.5)
    ch = round(H * 0.2)  # 102
    cw = round(W * 0.2)
    oh = np.floor(r[5] * np.float32(H + (1 - ch % 2))).astype(np.int32)
    ow = np.floor(r[6] * np.float32(W + (1 - cw % 2))).astype(np.int32)

    A = t * s
    BC = t * (np.float32(1.0) - s) / np.float32(3.0)
    GS = (np.float32(1.0) - t) / np.float32(3 * H * W)
    # the device computes out = A*(x + (BC/A)*m3 + C/A) with
    # C/A = (GS/A)*total + badd/A, so these constants are pre-divided by A
    pf = np.stack([A, BC / A, GS / A, badd / A], axis=1).astype(np.float32)  # [B,4]
    # fused element offset of the translated window within img[b, c]
    pi = ((th + PAD).astype(np.int64) * WP + (tw + PAD)).astype(np.int32)[
        :, None
    ]  # [B,1]

    idx = np.arange(H)
    a0 = np.maximum(0, oh - ch // 2)[:, None]
    a1 = np.minimum(H - 1, oh + (ch - ch // 2) - 1)[:, None]
    b0 = np.maximum(0, ow - cw // 2)[:, None]
    b1 = np.minimum(W - 1, ow + (cw - cw // 2) - 1)[:, None]
    rowz = (idx[None, :] >= a0) & (idx[None, :] <= a1)  # [B,H]
    colz = (idx[None, :] >= b0) & (idx[None, :] <= b1)  # [B,W]
    # even window start so the dynamic fp16 column slice stays 4B-aligned
    # (keeps the DVE cutout multiply in 2x perf mode)
    pcs0 = np.minimum(b0[:, 0], W - CW)
    pcs = (pcs0 - (pcs0 % 2)).astype(np.int32)[:, None]  # [B,1]
    # inverse cutout mask on the CW-wide window starting at pcs
    wi = pcs + np.arange(CW)[None, :]  # [B,CW]
    colz_win = np.take_along_axis(colz, wi, axis=1)  # [B,CW]
    invw = (
        1.0 - rowz[:, :, None] * colz_win[:, None, :]
    ).astype(np.float16)  # [B,H,CW]

    imp = np.zeros((B, C, HP, WP), dtype=np.float16)
    imp[:, :, PAD : PAD + H, PAD : PAD + W] = images
    return imp, pf, pi, pcs, invw


def _run(images, rand01, trace=False):
    images = np.ascontiguousarray(np.asarray(images, dtype=np.float32))
    imp, pf, pi, pcs, invw = _host_params(images, rand01)
    nc = _get_program()
    in_maps = [
        {
            "img": np.ascontiguousarray(imp[k * BS : (k + 1) * BS]),
            "pf": np.ascontiguousarray(pf[k * BS : (k + 1) * BS]),
            "pi": np.ascontiguousarray(pi[k * BS : (k + 1) * BS]),
            "pcs": np.ascontiguousarray(pcs[k * BS : (k + 1) * BS]),
            "invw": np.ascontiguousarray(invw[k * BS : (k + 1) * BS]),
        }
        for k in range(M)
    ]
    res = run_bass_kernel_spmd(nc, in_maps, list(range(M)), trace=trace)
    full = np.concatenate(
        [np.asarray(res.results[k]["out"], dtype=np.float32) for k in range(M)],
        axis=0,
    )
    return full, res


def kernel(images, rand01):
    full, _ = _run(images, rand01, trace=False)
    return full



# revision 16
# speedup vs baseline: 1.2975x; 1.1819x over previous
"""Trainium2 Bass kernel for DiscriminatorAugment (translation + color jitter +
cutout), data-parallel over 8 NeuronCores (8 samples each).

Math: with x0 = translated image, the reference's color jitter chain
    x1 = x0 + badd;  x2 = (x1 - mean_c x1)*s + mean_c x1;
    x3 = (x2 - mean_chw x2)*t + mean_chw x2
collapses to the per-pixel affine
    x3 = A*x0 + BC*m3 + C,   A = t*s, BC = t*(1-s)/3, m3 = sum_c x0,
    C = (1-t)*g0 + badd,     g0 = (sum_chw x0)/(3*H*W)
and cutout multiplies by (1 - rowmask*colmask).

Device work per sample (software-pipelined load(b) | m3(b-1) | out(b-2)):
dynamic-offset DMA load of the shifted window from a zero-padded copy of the
input (= translation, channels split across both HWDGE queues), DVE adds +
fused row-sum for m3, PE matmul with ones for the cross-partition sum
broadcast, ACT for C and the D = BC*m3 + C tile, DVE scalar_tensor_tensor
for A*x + D, a CW-wide dynamically-positioned window multiply for cutout
(alternating DVE/GpSimd), stores mostly via GpSimd/SWDGE so the load queues
never stall behind compute waits. HW exec ~157-165us/core vs a ~134us
DMA floor (52MB at the ~390GB/s per-core HBM ceiling).
"""
import threading

import numpy as np

import concourse.bass as bass
import concourse.mybir as mybir
import concourse.tile as tile
from concourse.bass_utils import run_bass_kernel_spmd

M = 8          # cores
B = 64         # full batch
BS = B // M    # samples per core
C, H, W = 3, 512, 512
PAD = 64       # translation margin (delta_h = delta_w = 64)
HP, WP = H + 2 * PAD, W + 2 * PAD
P = 128
NJ = H // P    # 4 row-chunks of 128
CH = round(H * 0.2)   # 102 cutout rows
CW = 106              # static cutout column window, even start (covers any
                      # clipped range even after rounding the start down)
F32 = mybir.dt.float32
F16 = mybir.dt.float16
I32 = mybir.dt.int32

# pf columns
I_A, I_BC, I_GS, I_BADD = 0, 1, 2, 3


def _split_waits(nc, max_waits=1):
    """Walrus in this container rejects >2 sem waits on one instruction
    ("Too many sync wait commands"). Hoist excess waits onto standalone
    single-wait event-semaphore instructions immediately before, same
    engine — semantics identical (waits execute before the instruction
    in program order either way)."""
    uid = 0
    for f in nc.m.functions:
        for bb in f.blocks:
            new_list, changed = [], False
            for inst in bb.instructions:
                si = inst.sync_info
                waits = list(si.on_wait) if si and si.on_wait else []
                if len(waits) > max_waits:
                    changed = True
                    for w in waits[:-max_waits]:
                        uid += 1
                        ev = mybir.InstEventSemaphore(name=f"splitwait_{uid}")
                        ev.engine = inst.engine
                        ev.sync_info = mybir.SyncInfo(on_wait=[w], on_update=[])
                        new_list.append(ev)
                    inst.sync_info = mybir.SyncInfo(
                        on_wait=waits[-max_waits:],
                        on_update=list(si.on_update) if si.on_update else [],
                    )
                new_list.append(inst)
            if changed:
                bb.instructions = new_list


def _bcast_part(ap, p=P):
    """Replicate a DRAM AP across p partitions (0-stride partition dim)."""
    return bass.AP(tensor=ap.tensor, offset=ap.offset, ap=[[0, p]] + list(ap.ap))


def _build_program():
    nc = bass.Bass(num_swdge_queues=4)
    img = nc.declare_dram_parameter("img", [BS, C, HP, WP], F16, isOutput=False)
    pf = nc.declare_dram_parameter("pf", [BS, 4], F32, isOutput=False)
    pi = nc.declare_dram_parameter("pi", [BS, 1], I32, isOutput=False)
    pcs = nc.declare_dram_parameter("pcs", [BS, 1], I32, isOutput=False)
    invw = nc.declare_dram_parameter("invw", [BS, H, CW], F16, isOutput=False)
    out = nc.declare_dram_parameter("out", [BS, C, H, W], F16, isOutput=True)

    Alu = mybir.AluOpType
    Act = mybir.ActivationFunctionType
    SP = mybir.EngineType.SP

    with tile.TileContext(nc) as tc:
        with (
            tc.tile_pool(name="work", bufs=8) as work,
            tc.tile_pool(name="singles", bufs=1) as singles,
            tc.tile_pool(name="psum", bufs=4, space="PSUM") as psum,
        ):
            # stage the dynamic offsets in SBUF first (everything else waits
            # on them): register loads from DRAM take ~2-3us on the issuing
            # engine, from SBUF they are cheap
            pi_sb = singles.tile([1, BS], I32)
            nc.sync.dma_start(out=pi_sb[:], in_=pi[:].rearrange("b one -> one b"))
            pcs_sb = singles.tile([1, BS], I32)
            nc.scalar.dma_start(out=pcs_sb[:], in_=pcs[:].rearrange("b one -> one b"))
            ones_t = singles.tile([P, P], F32)
            nc.vector.memset(ones_t[:], 1.0)
            ones16_t = singles.tile([P, 1], F16)
            nc.vector.memset(ones16_t[:], 1.0)
            pf_sb = singles.tile([P, BS, 4], F32)
            nc.gpsimd.dma_start(out=pf_sb[:], in_=_bcast_part(pf[:]))

            state = {}

            def stage_load(b):
                # alternate the HWDGE issuing engine for the dynamic loads:
                # each engine's register file only fits ~half the samples'
                # dynamic-offset expressions
                ld_eng_t = SP if b % 2 == 0 else mybir.EngineType.Activation
                ld_eng = nc.sync if b % 2 == 0 else nc.scalar
                x_t = work.tile([P, C, NJ, W], F16, tag="x")
                invw_t = work.tile([P, 1, NJ, CW], F16, tag="invw")
                # translated window load (dynamic element offset from pi:
                # pi[b] = r0*WP + c0), channels split across BOTH HWDGE
                # engines/queues so each sample's load latency halves.
                # During the fill the SWDGE store queue is idle, so the
                # first two samples also use it for the middle channel.
                engines = [SP, mybir.EngineType.Activation]
                if b < 2:
                    engines.append(mybir.EngineType.Pool)
                off = nc.values_load(
                    pi_sb[0:1, b : b + 1],
                    engines=engines,
                    min_val=0,
                    max_val=(HP - H) * WP + (WP - W),
                    skip_runtime_bounds_check=True,
                )
                for c in range(C):
                    base = img[b, c]
                    src = bass.AP(
                        tensor=base.tensor,
                        offset=base.offset + off,
                        ap=[[WP, P], [P * WP, NJ], [1, W]],
                    )
                    if c == 1:
                        eng = nc.gpsimd if b < 2 else ld_eng
                    else:
                        eng = nc.scalar if b % 2 == 0 else nc.sync
                    eng.dma_start(out=x_t[:, c], in_=src)
                (nc.gpsimd if b < 2 else ld_eng).dma_start(
                    out=invw_t[:, 0],
                    in_=invw[b].rearrange("(j p) w -> p j w", p=P),
                )
                state[b] = dict(x_t=x_t, invw_t=invw_t)

            def stage_m3(b):
                st = state[b]
                x_t = st["x_t"]
                m3_t = work.tile([P, 1, NJ, W], F16, tag="m3")
                c_t = work.tile([P, 1], F32, tag="c")
                tot_t = work.tile([1, 1], F32, tag="tot")
                cs_t = psum.tile([1, W], F32, tag="cs")
                g_t = psum.tile([P, 1], F32, tag="g")
                # m3 = x0+x1+x2 (2x mode tensor_tensor adds on raw x)
                nc.vector.tensor_tensor(
                    m3_t[:, 0], x_t[:, 0], x_t[:, 1], Alu.add
                )
                nc.vector.tensor_tensor(
                    m3_t[:, 0], m3_t[:, 0], x_t[:, 2], Alu.add
                )
                # global sum of m3 without touching DVE: PE column-sums the
                # four 512-wide blocks into one accumulating PSUM row, ACT
                # mini-reduces that row to a scalar, PE broadcasts it back
                # across all 128 partitions
                for k in range(NJ):
                    nc.tensor.matmul(
                        cs_t[:],
                        ones16_t[:],
                        m3_t[:, 0, k],
                        start=(k == 0),
                        stop=(k == NJ - 1),
                    )
                nc.scalar.activation(
                    cs_t[:], cs_t[:], Act.Identity, accum_out=tot_t[:]
                )
                nc.tensor.matmul(
                    g_t[:], ones_t[0:1, :], tot_t[:], start=True, stop=True
                )
                # C' = (GS/A) * total + badd/A   (per-partition [P,1])
                nc.scalar.activation(
                    c_t[:],
                    g_t[:],
                    Act.Identity,
                    bias=pf_sb[:, b, I_BADD : I_BADD + 1],
                    scale=pf_sb[:, b, I_GS : I_GS + 1],
                )
                # u' = (BC/A)*m3 + C'  (in place over m3), so that
                # A*(x + u') = A*x + BC*m3 + C exactly as the reference
                nc.scalar.activation(
                    m3_t[:, 0],
                    m3_t[:, 0],
                    Act.Identity,
                    bias=c_t[:],
                    scale=pf_sb[:, b, I_BC : I_BC + 1],
                )
                st["m3_t"] = m3_t
            def stage_out(b):
                st = state.pop(b)
                x_t, m3_t, invw_t = st["x_t"], st["m3_t"], st["invw_t"]
                # cutout mask multiply covers a CW-wide window at dynamic
                # start cs (host: min(b0, W-CW) rounded down to even, always
                # covers the rectangle). All on DVE: gpsimd runs its
                # tensor ops 4-10x slower and they'd gate the stores.
                cs = nc.values_load(
                    pcs_sb[0:1, b : b + 1],
                    engines=[mybir.EngineType.DVE],
                    min_val=0,
                    max_val=W - CW,
                    skip_runtime_bounds_check=True,
                )
                # y = x + u', one 2x-mode tensor_tensor over all three
                # channels with u' broadcast across the channel dim
                nc.vector.tensor_tensor(
                    x_t[:],
                    x_t[:],
                    m3_t[:].broadcast_to([P, C, NJ, W]),
                    Alu.add,
                )
                # out = A*y, plain tensor_scalar (4x perf mode)
                nc.vector.tensor_scalar(
                    out=x_t[:],
                    in0=x_t[:],
                    scalar1=pf_sb[:, b, I_A : I_A + 1],
                    scalar2=None,
                    op0=Alu.mult,
                )
                xwin = x_t[:, :, :, bass.ds(cs, CW)]
                nc.vector.tensor_tensor(
                    xwin, xwin, invw_t[:].broadcast_to([P, C, NJ, CW]), Alu.mult
                )
                # stores: two channels on gpsimd (SWDGE q0, never blocks the
                # load engines), one on an alternating HWDGE queue for
                # byte balance. Last two samples spread across all three.
                for c in range(C):
                    if b >= BS - 2:
                        st_eng = (nc.gpsimd, nc.sync, nc.scalar)[(b + c) % 3]
                    elif c == 2:
                        st_eng = nc.scalar if b % 2 == 0 else nc.sync
                    else:
                        st_eng = nc.gpsimd
                    st_eng.dma_start(
                        out=out[b, c].rearrange("(j p) w -> p j w", p=P),
                        in_=x_t[:, c],
                    )

            # software-pipelined emission: load(b) | m3(b-1) | out(b-2) so
            # the scheduler interleaves sample b+1's DVE work into sample
            # b's PE/ACT latency chain
            for i in range(BS + 2):
                if i < BS:
                    stage_load(i)
                if 0 <= i - 1 < BS:
                    stage_m3(i - 1)
                if 0 <= i - 2 < BS:
                    stage_out(i - 2)

    _split_waits(nc)
    return nc


_cache = threading.local()


def _get_program():
    nc = getattr(_cache, "nc", None)
    if nc is None:
        nc = _build_program()
        _cache.nc = nc
    return nc


def _host_params(images, rand01):
    """Per-sample parameters, computed with float32 semantics matching the
    jax reference."""
    r = np.asarray(rand01, dtype=np.float32).reshape(7, B)
    th = np.floor(r[0] * np.float32(2 * PAD + 1)).astype(np.int32) - PAD
    tw = np.floor(r[1] * np.float32(2 * PAD + 1)).astype(np.int32) - PAD
    badd = r[2] - np.float32(0.5)
    s = r[3] * np.float32(2.0)
    t = r[4] + np.float32(0.5)
    ch = round(H * 0.2)  # 102
    cw = round(W * 0.2)
    oh = np.floor(r[5] * np.float32(H + (1 - ch % 2))).astype(np.int32)
    ow = np.floor(r[6] * np.float32(W + (1 - cw % 2))).astype(np.int32)

    A = t * s
    BC = t * (np.float32(1.0) - s) / np.float32(3.0)
    GS = (np.float32(1.0) - t) / np.float32(3 * H * W)
    # the device computes out = A*(x + (BC/A)*m3 + C/A) with
    # C/A = (GS/A)*total + badd/A, so these constants are pre-divided by A
    pf = np.stack([A, BC / A, GS / A, badd / A], axis=1).astype(np.float32)  # [B,4]
    # fused element offset of the translated window within img[b, c]
    pi = ((th + PAD).astype(np.int64) * WP + (tw + PAD)).astype(np.int32)[
        :, None
    ]  # [B,1]

    idx = np.arange(H)
    a0 = np.maximum(0, oh - ch // 2)[:, None]
    a1 = np.minimum(H - 1, oh + (ch - ch // 2) - 1)[:, None]
    b0 = np.maximum(0, ow - cw // 2)[:, None]
    b1 = np.minimum(W - 1, ow + (cw - cw // 2) - 1)[:, None]
    rowz = (idx[None, :] >= a0) & (idx[None, :] <= a1)  # [B,H]
    colz = (idx[None, :] >= b0) & (idx[None, :] <= b1)  # [B,W]
    # even window start so the dynamic fp16 column slice stays 4B-aligned
    # (keeps the DVE cutout multiply in 2x perf mode)
    pcs0 = np.minimum(b0[:, 0], W - CW)
    pcs = (pcs0 - (pcs0 % 2)).astype(np.int32)[:, None]  # [B,1]
    # inverse cutout mask on the CW-wide window starting at pcs
    wi = pcs + np.arange(CW)[None, :]  # [B,CW]
    colz_win = np.take_along_axis(colz, wi, axis=1)  # [B,CW]
    invw = (
        1.0 - rowz[:, :, None] * colz_win[:, None, :]
    ).astype(np.float16)  # [B,H,CW]

    imp = np.zeros((B, C, HP, WP), dtype=np.float16)
    imp[:, :, PAD : PAD + H, PAD : PAD + W] = images
    return imp, pf, pi, pcs, invw


def _run(images, rand01, trace=False):
    images = np.ascontiguousarray(np.asarray(images, dtype=np.float32))
    imp, pf, pi, pcs, invw = _host_params(images, rand01)
    nc = _get_program()
    in_maps = [
        {
            "img": np.ascontiguousarray(imp[k * BS : (k + 1) * BS]),
            "pf": np.ascontiguousarray(pf[k * BS : (k + 1) * BS]),
            "pi": np.ascontiguousarray(pi[k * BS : (k + 1) * BS]),
            "pcs": np.ascontiguousarray(pcs[k * BS : (k + 1) * BS]),
            "invw": np.ascontiguousarray(invw[k * BS : (k + 1) * BS]),
        }
        for k in range(M)
    ]
    res = run_bass_kernel_spmd(nc, in_maps, list(range(M)), trace=trace)
    full = np.concatenate(
        [np.asarray(res.results[k]["out"], dtype=np.float32) for k in range(M)],
        axis=0,
    )
    return full, res


def kernel(images, rand01):
    full, _ = _run(images, rand01, trace=False)
    return full

